# revision 11
# baseline (speedup 1.0000x reference)
"""COCOA loss kernel for 8 Trainium2 NeuronCores.

loss = SCALE_LOSS * sum_b pos[b] + LAMBDA * sum(neg)
  pos[b] = mean_{v,w} exp((1 - zn[v,b]·zn[w,b]) / T)           (per-sample view gram)
  neg    = sum_{v,b,c!=b} exp(zn[v,b]·zn[v,c] / T) / (B-1)     (per-view batch gram)

Device strategy (SPMD, one program, per-core data):
  * Host normalizes z (0.05% of FLOPs), transposes to [V, D, B] bf16 and
    rolls columns by 512*core so each core's 512 sample rows sit at local
    columns [0, 512).
  * Per view the global [B, B] gram splits into an 8x8 grid of 512x512
    blocks. Core c computes blocks (c, c+d mod 8) for d = 0..4 using the
    TensorE; since exp(G)^T = exp(G^T) elementwise and G is symmetric,
    host counts d in {1,2,3} twice, d in {0,4} once -> every ordered
    off-diagonal pair counted exactly once with 37.5% less work.
  * ScalarE evaluates exp with the fused free-dim accumulator
    (activation(..., accum_out=)) directly from PSUM; only [128,1] column
    sums leave the engines.
  * The true diagonal exp(sim_bb/T) ~= e^2 is subtracted analytically on
    host (error ~1e-8 relative).
  * pos term: VectorE multiply + free-dim reduce over the core's own rows
    in natural layout + ScalarE exp-accumulate.
"""

import sys

import numpy as np

try:
    import concourse.bass as bass  # noqa: F401
except ImportError:  # pragma: no cover
    sys.path.insert(0, "/opt/trn_rl_repo")

import concourse.bass as bass
import concourse.bacc as bacc
import concourse.mybir as mybir
import concourse.tile as tile
from concourse.bass_utils import run_bass_kernel_spmd

import ml_dtypes

BF16 = ml_dtypes.bfloat16

# Problem constants (hardcoded per the harness contract).
B = 4096          # batch
V = 6             # views
D = 256           # embedding dim
KC = 2            # contraction chunks of 128 (D = 256)
NCORE = 8
BLK = B // NCORE  # 512 rows per core
MT = BLK // 128   # 4 m-tiles per core
ND = 5            # d-blocks computed per core (d = 0..4)

TEMPERATURE = 0.5
SCALE_LOSS = 1.0 / 32.0
LAMBDA = 0.0039

# stats layout: [128, 48] fp32
#   col v*6 + 0      : d0 (diagonal block) group sum, weight x1
#   col v*6 + 1 + m  : d1..d3 group sum for m-tile m, weight x2
#   col v*6 + 5      : d4 group sum, weight x1
#   col 36 + 2t      : pos cross-view pair sums (v<w), b-tile t
#   col 36 + 2t + 1  : pos self-view sums (v==w), b-tile t
NEG_COLS = V * 6          # 36
NSTAT = 48

F32 = mybir.dt.float32
BF16_DT = mybir.dt.bfloat16

_PAIRS = [(v, w) for v in range(V) for w in range(v + 1, V)]  # 15
_SELF = [(v, v) for v in range(V)]                            # 6


def _build_nc(reps: int = 1) -> bass.Bass:
    """reps > 1 repeats the whole compute body (inputs stay resident) so a
    timing harness can measure steady-state HW time differentially."""
    nc = bacc.Bacc("TRN2", debug=False, num_devices=NCORE)

    zt_d = nc.dram_tensor("zt", [V, KC, 128, B], BF16_DT, kind="ExternalInput")
    zb_d = nc.dram_tensor("zb", [MT, 128, V * D], BF16_DT, kind="ExternalInput")
    st_d = nc.dram_tensor("stats", [128, NSTAT], F32, kind="ExternalOutput")

    with tile.TileContext(nc) as tc:
        with (
            tc.tile_pool(name="ztp", bufs=1) as ztp,
            tc.tile_pool(name="zbp", bufs=1) as zbp,
            tc.tile_pool(name="stp", bufs=1) as stp,
            tc.tile_pool(name="simsp", bufs=2) as simsp,
            tc.tile_pool(name="prodp", bufs=2) as prodp,
            tc.tile_pool(name="pexpp", bufs=2) as pexpp,
            tc.tile_pool(name="escrp", bufs=2) as escrp,
            tc.tile_pool(name="psump", bufs=2, space="PSUM") as psump,
        ):
            stats = stp.tile([128, NSTAT], F32)

            # ---- DMA inputs in ----
            zt_sb = [[ztp.tile([128, B], BF16_DT, tag=f"zt_{v}_{k}", name=f"zt_{v}_{k}")
                      for k in range(KC)] for v in range(V)]
            for v in range(V):
                for k in range(KC):
                    nc.sync.dma_start(zt_sb[v][k][:, :], zt_d.ap()[v, k])
            zb_sb = [zbp.tile([128, V * D], BF16_DT, tag=f"zb_{t}", name=f"zb_{t}")
                     for t in range(MT)]
            for t in range(MT):
                nc.sync.dma_start(zb_sb[t][:, :], zb_d.ap()[t])

            for _rep in range(reps):
                self_rep = _rep  # noqa: F841
                run_body(nc, tc, zt_sb, zb_sb, stats,
                         simsp, prodp, pexpp, escrp, psump)

            # ---- stats out ----
            nc.sync.dma_start(st_d.ap()[:, :], stats[:, :])

    nc.compile()
    return nc


def run_body(nc, tc, zt_sb, zb_sb, stats, simsp, prodp, pexpp, escrp, psump):
    # ---- pos term: per-sample cross-view sims on DVE ----
    for t in range(MT):
        sims = simsp.tile([128, 21], F32, tag="sims", name="sims")
        for j, (v, w) in enumerate(_PAIRS + _SELF):
            prod = prodp.tile([128, D], BF16_DT, tag="prod", name="prod")
            nc.vector.tensor_mul(
                prod[:, :],
                zb_sb[t][:, v * D:(v + 1) * D],
                zb_sb[t][:, w * D:(w + 1) * D],
            )
            nc.vector.tensor_reduce(
                sims[:, j:j + 1], prod[:, :],
                axis=mybir.AxisListType.X, op=mybir.AluOpType.add,
            )
        pexp = pexpp.tile([128, 21], BF16_DT, tag="pexp", name="pexp")
        # exp((1 - s)/T) = e^2 * exp(-2 s); the e^2 factor is applied
        # on host (no 2.0 const AP is pre-registered)
        nc.scalar.activation(
            pexp[:, 0:15], sims[:, 0:15],
            mybir.ActivationFunctionType.Exp,
            bias=0.0, scale=-2.0,
            accum_out=stats[:, NEG_COLS + 2 * t: NEG_COLS + 2 * t + 1],
        )
        nc.scalar.activation(
            pexp[:, 15:21], sims[:, 15:21],
            mybir.ActivationFunctionType.Exp,
            bias=0.0, scale=-2.0,
            accum_out=stats[:, NEG_COLS + 2 * t + 1: NEG_COLS + 2 * t + 2],
        )

    # ---- neg term: block grams on PE, exp+sum on ACT ----
    def gram_group(v: int, chunks, stat_col: int):
        """chunks: list of (m, dloc) pairs; each is a [128,512] matmul
        output packed side by side into one PSUM group, followed by
        one fused exp+accumulate over the whole group."""
        fd = 512 * len(chunks)
        ps = psump.tile([128, fd], F32, tag="gram", name="gram")
        for i, (m, dloc) in enumerate(chunks):
            for k in range(KC):
                nc.tensor.matmul(
                    ps[:, i * 512:(i + 1) * 512],
                    zt_sb[v][k][:, m * 128:(m + 1) * 128],
                    zt_sb[v][k][:, dloc * 512:(dloc + 1) * 512],
                    start=(k == 0),
                    stop=(k == KC - 1),
                )
        esc = escrp.tile([128, 2048], BF16_DT, tag="esc", name="esc")
        # exp(s / T) = exp(2 s)
        nc.scalar.activation(
            esc[:, 0:fd], ps[:, 0:fd],
            mybir.ActivationFunctionType.Exp,
            bias=0.0, scale=2.0,
            accum_out=stats[:, stat_col:stat_col + 1],
        )

    for v in range(V):
        # d0: the core's diagonal block, all 4 m-tiles, weight x1
        gram_group(v, [(m, 0) for m in range(MT)], v * 6 + 0)
        # d1..d3 per m-tile, weight x2
        for m in range(MT):
            gram_group(v, [(m, d) for d in (1, 2, 3)], v * 6 + 1 + m)
        # d4, all 4 m-tiles, weight x1
        gram_group(v, [(m, 4) for m in range(MT)], v * 6 + 5)


_NC_CACHE = None


def _get_nc() -> bass.Bass:
    global _NC_CACHE
    if _NC_CACHE is None:
        _NC_CACHE = _build_nc()
    return _NC_CACHE


def _prep_inputs(z: np.ndarray) -> list[dict[str, np.ndarray]]:
    z = np.asarray(z, dtype=np.float32)
    zn = z / np.linalg.norm(z, axis=-1, keepdims=True)          # [B, V, D] f32
    zT = np.ascontiguousarray(zn.transpose(1, 2, 0))            # [V, D, B]
    zt_bf = zT.reshape(V, KC, 128, B).astype(BF16)
    in_maps = []
    for c in range(NCORE):
        zt_c = np.roll(zt_bf, -BLK * c, axis=-1)
        zb_c = np.ascontiguousarray(
            zn[BLK * c:BLK * (c + 1)].reshape(MT, 128, V * D)
        ).astype(BF16)
        in_maps.append({"zt": np.ascontiguousarray(zt_c), "zb": zb_c})
    return in_maps


def _host_reduce(stats_list) -> np.float32:
    e2 = float(np.exp(2.0))
    neg_sum = 0.0
    pos_sum = 0.0
    for c in range(NCORE):
        st = np.asarray(stats_list[c], dtype=np.float64)
        for v in range(V):
            neg_sum += st[:, v * 6 + 0].sum()          # d0, x1
            neg_sum += 2.0 * st[:, v * 6 + 1: v * 6 + 5].sum()  # d1..3, x2
            neg_sum += st[:, v * 6 + 5].sum()          # d4, x1
        neg_sum -= V * BLK * e2                        # analytic diagonal
        pairs = st[:, NEG_COLS:NEG_COLS + 2 * MT:2].sum()
        selfs = st[:, NEG_COLS + 1:NEG_COLS + 2 * MT + 1:2].sum()
        pos_sum += e2 * (2.0 * pairs + selfs) / (V * V)
    total = SCALE_LOSS * pos_sum + LAMBDA * neg_sum / (B - 1)
    return np.float32(total)


def run(z: np.ndarray, trace: bool = False):
    """Returns (loss, BassKernelResults)."""
    nc = _get_nc()
    in_maps = _prep_inputs(z)
    res = run_bass_kernel_spmd(
        nc, in_maps, core_ids=list(range(NCORE)), trace=trace
    )
    stats_list = [res.results[c]["stats"] for c in range(NCORE)]
    return _host_reduce(stats_list), res


def kernel(z: np.ndarray) -> np.ndarray:
    loss, _ = run(z, trace=False)
    return np.asarray(loss, dtype=np.float32)


# revision 12
# speedup vs baseline: 3.8378x; 3.8378x over previous
"""COCOA loss kernel for 8 Trainium2 NeuronCores.

loss = SCALE_LOSS * sum_b pos[b] + LAMBDA * sum(neg)
  pos[b] = mean_{v,w} exp((1 - zn[v,b]·zn[w,b]) / T)           (per-sample view gram)
  neg    = sum_{v,b,c!=b} exp(zn[v,b]·zn[v,c] / T) / (B-1)     (per-view batch gram)

Device strategy (SPMD, one program, per-core data):
  * Host normalizes z (0.05% of FLOPs), transposes to [V, D, B] bf16 and
    rolls columns by 512*core so each core's 512 sample rows sit at local
    columns [0, 512).
  * Per view the global [B, B] gram splits into an 8x8 grid of 512x512
    blocks. Core c computes blocks (c, c+d mod 8) for d = 0..4 using the
    TensorE; since exp(G)^T = exp(G^T) elementwise and G is symmetric,
    host counts d in {1,2,3} twice, d in {0,4} once -> every ordered
    off-diagonal pair counted exactly once with 37.5% less work.
  * ScalarE evaluates exp with the fused free-dim accumulator
    (activation(..., accum_out=)) directly from PSUM; only [128,1] column
    sums leave the engines.
  * The true diagonal exp(sim_bb/T) ~= e^2 is subtracted analytically on
    host (error ~1e-8 relative).
  * pos term: VectorE multiply + free-dim reduce over the core's own rows
    in natural layout + ScalarE exp-accumulate.
"""

import sys

import numpy as np

try:
    import concourse.bass as bass  # noqa: F401
except ImportError:  # pragma: no cover
    sys.path.insert(0, "/opt/trn_rl_repo")

import concourse.bass as bass
import concourse.bacc as bacc
import concourse.mybir as mybir
import concourse.tile as tile
from concourse.bass_utils import run_bass_kernel_spmd

import ml_dtypes

BF16 = ml_dtypes.bfloat16

# Problem constants (hardcoded per the harness contract).
B = 4096          # batch
V = 6             # views
D = 256           # embedding dim
KC = 2            # contraction chunks of 128 (D = 256)
NCORE = 8
BLK = B // NCORE  # 512 rows per core
MT = BLK // 128   # 4 m-tiles per core
ND = 5            # d-blocks computed per core (d = 0..4)

TEMPERATURE = 0.5
SCALE_LOSS = 1.0 / 32.0
LAMBDA = 0.0039

# stats layout: [128, 72] fp32. Per view v, 10 cols at base v*10 — one per
# [128, 1024] PSUM group (2 chunks x 512):
#   +0, +1       : d0 (diagonal block), m-pairs (0,1) and (2,3), weight x1
#   +2 .. +7     : d in {1,2,3} x m-pairs (0,1)/(2,3), weight x2
#   +8, +9       : d4, m-pairs (0,1)/(2,3), weight x1
#   col 60 + 2t  : pos cross-view pair sums (v<w), b-tile t
#   col 60 + 2t+1: pos self-view sums (v==w), b-tile t
NEG_COLS = V * 10         # 60
NSTAT = 72

F32 = mybir.dt.float32
BF16_DT = mybir.dt.bfloat16

_PAIRS = [(v, w) for v in range(V) for w in range(v + 1, V)]  # 15
_SELF = [(v, v) for v in range(V)]                            # 6


def _build_nc(reps: int = 1) -> bass.Bass:
    """reps > 1 repeats the whole compute body (inputs stay resident) so a
    timing harness can measure steady-state HW time differentially."""
    nc = bacc.Bacc("TRN2", debug=False, num_devices=NCORE)

    zt_d = nc.dram_tensor("zt", [V, KC, 128, B], BF16_DT, kind="ExternalInput")
    zb_d = nc.dram_tensor("zb", [MT, 128, V * D], BF16_DT, kind="ExternalInput")
    st_d = nc.dram_tensor("stats", [128, NSTAT], F32, kind="ExternalOutput")

    with tile.TileContext(nc) as tc:
        with (
            tc.tile_pool(name="ztp", bufs=1) as ztp,
            tc.tile_pool(name="zbp", bufs=1) as zbp,
            tc.tile_pool(name="stp", bufs=1) as stp,
            tc.tile_pool(name="simsp", bufs=2) as simsp,
            tc.tile_pool(name="prodp", bufs=2) as prodp,
            tc.tile_pool(name="pexpp", bufs=2) as pexpp,
            tc.tile_pool(name="escrp", bufs=2) as escrp,
            tc.tile_pool(name="psump", bufs=4, space="PSUM") as psump,
        ):
            stats = stp.tile([128, NSTAT], F32)

            # ---- DMA inputs in ----
            zt_sb = [[ztp.tile([128, B], BF16_DT, tag=f"zt_{v}_{k}", name=f"zt_{v}_{k}")
                      for k in range(KC)] for v in range(V)]
            for v in range(V):
                for k in range(KC):
                    nc.sync.dma_start(zt_sb[v][k][:, :], zt_d.ap()[v, k])
            zb_sb = [zbp.tile([128, V * D], BF16_DT, tag=f"zb_{t}", name=f"zb_{t}")
                     for t in range(MT)]
            for t in range(MT):
                nc.sync.dma_start(zb_sb[t][:, :], zb_d.ap()[t])

            for _rep in range(reps):
                self_rep = _rep  # noqa: F841
                run_body(nc, tc, zt_sb, zb_sb, stats,
                         simsp, prodp, pexpp, escrp, psump)

            # ---- stats out ----
            nc.sync.dma_start(st_d.ap()[:, :], stats[:, :])

    nc.compile()
    return nc


def run_body(nc, tc, zt_sb, zb_sb, stats, simsp, prodp, pexpp, escrp, psump):
    # ---- neg term: block grams on PE, exp+sum on ACT ----
    def gram_group(v: int, chunks, stat_col: int):
        """chunks: two (m, dloc) pairs; each is a [128,512] matmul output
        packed side by side into one 2-bank PSUM group, followed by one
        fused exp+accumulate over the whole group. Uniform [128,1024]
        groups x bufs=4 keep 4 groups in flight for PE/ACT overlap."""
        fd = 512 * len(chunks)
        ps = psump.tile([128, fd], F32, tag="gram", name="gram")
        for i, (m, dloc) in enumerate(chunks):
            for k in range(KC):
                nc.tensor.matmul(
                    ps[:, i * 512:(i + 1) * 512],
                    zt_sb[v][k][:, m * 128:(m + 1) * 128],
                    zt_sb[v][k][:, dloc * 512:(dloc + 1) * 512],
                    start=(k == 0),
                    stop=(k == KC - 1),
                )
        esc = escrp.tile([128, 1024], BF16_DT, tag="esc", name="esc")
        # exp(s / T) = exp(2 s)
        nc.scalar.activation(
            esc[:, 0:fd], ps[:, 0:fd],
            mybir.ActivationFunctionType.Exp,
            bias=0.0, scale=2.0,
            accum_out=stats[:, stat_col:stat_col + 1],
        )

    for v in range(V):
        col = v * 10
        for mp in ((0, 1), (2, 3)):                      # d0, weight x1
            gram_group(v, [(m, 0) for m in mp], col)
            col += 1
        for d in (1, 2, 3):                              # weight x2
            for mp in ((0, 1), (2, 3)):
                gram_group(v, [(m, d) for m in mp], col)
                col += 1
        for mp in ((0, 1), (2, 3)):                      # d4, weight x1
            gram_group(v, [(m, 4) for m in mp], col)
            col += 1

    # ---- pos term: per-sample cross-view sims on DVE (after the neg
    # phase so the pos ACT ops sit at the tail of ACT's queue and never
    # head-block the exp groups) ----
    for t in range(MT):
        prods = prodp.tile([128, 21, D], BF16_DT, tag="prods", name="prods")
        for j, (v, w) in enumerate(_PAIRS + _SELF):
            nc.vector.tensor_mul(
                prods[:, j, :],
                zb_sb[t][:, v * D:(v + 1) * D],
                zb_sb[t][:, w * D:(w + 1) * D],
            )
        sims = simsp.tile([128, 21], F32, tag="sims", name="sims")
        nc.vector.tensor_reduce(
            sims[:, :], prods[:, :, :],
            axis=mybir.AxisListType.X, op=mybir.AluOpType.add,
        )
        pexp = pexpp.tile([128, 21], BF16_DT, tag="pexp", name="pexp")
        # exp((1 - s)/T) = e^2 * exp(-2 s); the e^2 factor is applied
        # on host (no 2.0 const AP is pre-registered)
        nc.scalar.activation(
            pexp[:, 0:15], sims[:, 0:15],
            mybir.ActivationFunctionType.Exp,
            bias=0.0, scale=-2.0,
            accum_out=stats[:, NEG_COLS + 2 * t: NEG_COLS + 2 * t + 1],
        )
        nc.scalar.activation(
            pexp[:, 15:21], sims[:, 15:21],
            mybir.ActivationFunctionType.Exp,
            bias=0.0, scale=-2.0,
            accum_out=stats[:, NEG_COLS + 2 * t + 1: NEG_COLS + 2 * t + 2],
        )


_NC_CACHE = None


def _get_nc() -> bass.Bass:
    global _NC_CACHE
    if _NC_CACHE is None:
        _NC_CACHE = _build_nc()
    return _NC_CACHE


def _prep_inputs(z: np.ndarray) -> list[dict[str, np.ndarray]]:
    z = np.asarray(z, dtype=np.float32)
    zn = z / np.linalg.norm(z, axis=-1, keepdims=True)          # [B, V, D] f32
    zT = np.ascontiguousarray(zn.transpose(1, 2, 0))            # [V, D, B]
    zt_bf = zT.reshape(V, KC, 128, B).astype(BF16)
    in_maps = []
    for c in range(NCORE):
        zt_c = np.roll(zt_bf, -BLK * c, axis=-1)
        zb_c = np.ascontiguousarray(
            zn[BLK * c:BLK * (c + 1)].reshape(MT, 128, V * D)
        ).astype(BF16)
        in_maps.append({"zt": np.ascontiguousarray(zt_c), "zb": zb_c})
    return in_maps


def _host_reduce(stats_list) -> np.float32:
    e2 = float(np.exp(2.0))
    neg_sum = 0.0
    pos_sum = 0.0
    for c in range(NCORE):
        st = np.asarray(stats_list[c], dtype=np.float64)
        for v in range(V):
            neg_sum += st[:, v * 10 + 0: v * 10 + 2].sum()          # d0, x1
            neg_sum += 2.0 * st[:, v * 10 + 2: v * 10 + 8].sum()    # d1..3, x2
            neg_sum += st[:, v * 10 + 8: v * 10 + 10].sum()         # d4, x1
        neg_sum -= V * BLK * e2                        # analytic diagonal
        pairs = st[:, NEG_COLS:NEG_COLS + 2 * MT:2].sum()
        selfs = st[:, NEG_COLS + 1:NEG_COLS + 2 * MT + 1:2].sum()
        pos_sum += e2 * (2.0 * pairs + selfs) / (V * V)
    total = SCALE_LOSS * pos_sum + LAMBDA * neg_sum / (B - 1)
    return np.float32(total)


def run(z: np.ndarray, trace: bool = False):
    """Returns (loss, BassKernelResults)."""
    nc = _get_nc()
    in_maps = _prep_inputs(z)
    res = run_bass_kernel_spmd(
        nc, in_maps, core_ids=list(range(NCORE)), trace=trace
    )
    stats_list = [res.results[c]["stats"] for c in range(NCORE)]
    return _host_reduce(stats_list), res


def kernel(z: np.ndarray) -> np.ndarray:
    loss, _ = run(z, trace=False)
    return np.asarray(loss, dtype=np.float32)
